# revision 77
# baseline (speedup 1.0000x reference)
"""Additive (Bahdanau) attention on 8 TRN2 NeuronCores.

reference:
    q = query @ Wq.T + bq                  [B, Lq, H]
    k = key @ Wk.T + bk                    [B, Lk, H]
    scores[b,q,k] = sum_h v[h] * tanh(qp[b,q,h] + kp[b,k,h]) (+ bv)
    scores = where(mask==0, -inf, scores)
    attn_w = softmax(scores, axis=-1)      [B, Lq, Lk]
    attn_out = attn_w @ value              [B, Lq, H]
    returns (attn_out, attn_w)

Sharding: B*Lq = 1024 query rows split 8 ways -> 128 rows/core, each core
gets its batch's key/value/mask. Zero cross-core communication.

The workload is bound by ScalarE (ACT), the only transcendental engine:
Lq*Lk*H/8 = 16.8M tanh evaluations per core at 1 elem/lane/cycle @ 1.2
GHz = ~109 us. Everything else is pipelined under that stream.

Per-core dataflow (h on partitions for the tanh pipeline):
  - host packs transposed bf16 layouts (queryT/keyT/WqT/WkT concatenated
    into one array per DMA) — pure layout/rounding prep, all math stays
    on device; PE computes projections qpT[h,q], kpT[h,k] (f32 PSUM).
  - main loop over ramped q-blocks x h-tiles: DVE tensor_scalar_add
    broadcasts qpT[:,q] over kpT (bf16, 4x mode) -> big S_in tile;
    ScalarE tanh in [128, qb*512] instructions; PE reduces over h with
    the v column as moving operand, tanh tile as stationary (FWL bf16),
    giving transposed scores scT[k, q] in PSUM (PE output partition base
    must be 32-aligned, which forbids per-q score rows directly).
  - scores are built in two 64-row half tiles in different PSUM banks;
    each half's postprocessing (PSUM eviction fused with the mask bias
    via tensor_scalar — the mask is per-partition in the transposed
    layout — PE transposes to [q, k], masked softmax with exp accum_out
    row sums, attn_w DMA, PE re-transpose, attn_out matmul split in two
    h-halves) overlaps the other half's tanh stream. The softmax
    max-subtraction uses the static bound sum|v| >= |scores| (tanh < 1),
    so exp needs no row-max reduction; attn_out is computed from the
    unnormalized exp with 1/rowsum folded into the PSUM eviction.

Numerics: the tanh/score pipeline runs in bf16 (inputs rounded on host,
tanh output bf16, f32 PSUM accumulation); softmax and attn_out are f32.
End-to-end rel err ~5e-3. bv is mathematically irrelevant: softmax is
shift invariant and raw scores are not returned.
"""

from contextlib import ExitStack

import numpy as np

import concourse.bass as bass
import concourse.tile as tile
from concourse import bacc, mybir
from concourse.masks import make_identity

B, LQ, LK, H = 4, 256, 512, 256
NCORES = 8
QROWS = B * LQ // NCORES  # 128 query rows per core
NEG_BIG = -1e30

F32 = mybir.dt.float32
BF16 = mybir.dt.bfloat16
I32 = mybir.dt.int32
# dtype of the tanh pipeline (kpT/qpTb/sin/tb/v): bf16 gives DVE 4x mode
# on the broadcast adds and halves SBUF; scores accumulate in f32 PSUM.
TDT = BF16

_CACHE: dict = {}


def _build_nc():
    nc = bacc.Bacc()

    # host-side layout prep (make_in_maps): transposed + concatenated into
    # one wide array per DMA so the prologue is a handful of transfers.
    #   wt:   (WqT0 | WqT1 | WkT0 | WkT1)          [128, 4*H]
    #   qt:   (queryT0 | queryT1)                  [128, 2*QROWS]
    #   kt:   (keyT0 | keyT1)                      [128, 2*LK]
    #   vt:   (value[0:128] | ... | value[384:512])[128, 4*H]
    #   cols: (bq0|bq1|bk0|bk1|v0|v1)              [128, 6]
    wt = nc.declare_dram_parameter("wt", [128, 4 * H], TDT, isOutput=False)
    qt = nc.declare_dram_parameter("qt", [128, 2 * QROWS], TDT, isOutput=False)
    kt = nc.declare_dram_parameter("kt", [128, 2 * LK], TDT, isOutput=False)
    vt = nc.declare_dram_parameter("vt", [128, 4 * H], F32, isOutput=False)
    cols = nc.declare_dram_parameter("cols", [128, 6], F32, isOutput=False)
    mask = nc.declare_dram_parameter("mask", [LK], I32, isOutput=False)

    attn_out = nc.declare_dram_parameter("attn_out", [QROWS, H], F32, isOutput=True)
    attn_w = nc.declare_dram_parameter("attn_w", [QROWS, LK], F32, isOutput=True)

    HT = H // 128  # h tiles (2)
    IT = H // 128  # hin tiles (2)
    KT = LK // 128  # key row tiles (4)


    with tile.TileContext(nc) as tc, ExitStack() as ctx:
        persist = ctx.enter_context(tc.tile_pool(name="persist", bufs=1))
        sin_pool = ctx.enter_context(tc.tile_pool(name="sin", bufs=2))

        tp_ps = ctx.enter_context(tc.tile_pool(name="tp_ps", bufs=2, space="PSUM"))
        pj_ps = ctx.enter_context(tc.tile_pool(name="pj_ps", bufs=2, space="PSUM"))
        sc_ps = ctx.enter_context(tc.tile_pool(name="sc_ps", bufs=1, space="PSUM"))
        out_ps = ctx.enter_context(tc.tile_pool(name="out_ps", bufs=2, space="PSUM"))

        # ---- packed loads (one DMA each; critical path first) ----------
        # key halves + weights on three queues so all prologue-critical
        # bytes land in parallel; the ACT-queued half is issued before the
        # warm tanh so it isn't delayed by the activation-table load
        kt_sb = persist.tile([128, 2 * LK], TDT)
        nc.sync.dma_start(out=kt_sb[:, :LK], in_=kt[:, :LK])
        nc.scalar.dma_start(out=kt_sb[:, LK:], in_=kt[:, LK:])
        wt_sb = persist.tile([128, 4 * H], TDT)
        nc.gpsimd.dma_start(out=wt_sb, in_=wt[:, :])

        # ---- constants / identity / ACT table warmup -------------------
        ident = persist.tile([128, 128], F32)
        make_identity(nc, ident)
        warm = persist.tile([128, 1], F32)
        nc.vector.memset(warm, 0.0)
        # touch the exp_and_others table set early (covers tanh + exp)
        nc.scalar.activation(warm, warm, mybir.ActivationFunctionType.Tanh)
        cols_sb = persist.tile([128, 6], F32)
        nc.sync.dma_start(out=cols_sb, in_=cols[:, :])
        qt_sb = persist.tile([128, 2 * QROWS], TDT)
        nc.sync.dma_start(out=qt_sb, in_=qt[:, :])

        wqT = [wt_sb[:, it * H:(it + 1) * H] for it in range(IT)]
        wkT = [wt_sb[:, (2 + it) * H:(3 + it) * H] for it in range(IT)]
        queryT_sb = [qt_sb[:, it * QROWS:(it + 1) * QROWS] for it in range(IT)]
        keyT_sb = [kt_sb[:, it * LK:(it + 1) * LK] for it in range(IT)]
        bq_col = [cols_sb[:, ht:ht + 1] for ht in range(HT)]
        bk_col = [cols_sb[:, 2 + ht:3 + ht] for ht in range(HT)]

        # v columns: bf16 to match the tanh-tile lhsT dtype
        v_col = []
        for ht in range(HT):
            t = persist.tile([128, 1], TDT, tag=f"v{ht}")
            nc.vector.tensor_copy(t, cols_sb[:, 4 + ht:5 + ht])
            v_col.append(t)
        bsum = []
        for ht in range(HT):
            t = persist.tile([128, 1], F32, tag=f"bsum{ht}")
            nc.vector.tensor_add(t, bq_col[ht], bk_col[ht])
            bsum.append(t)

        # ---- projections ----------------------------------------------
        # qpT[ht][h', qrow] = sum_hin Wq[h, hin] * queryT[hin, qrow]  (+bq+bk)
        # kpT first: it is the long pole into the first tanh block.
        # ScalarE does the PSUM->SBUF copies (ACT is idle in the prologue,
        # DVE is busy with the first adds).
        kpT = []
        for ht in range(HT):
            pk = pj_ps.tile([128, LK], F32, tag="pj")
            for it in range(IT):
                nc.tensor.matmul(
                    pk, lhsT=wkT[it][:, ht * 128:(ht + 1) * 128], rhs=keyT_sb[it],
                    start=(it == 0), stop=(it == IT - 1),
                )
            t = persist.tile([128, LK], TDT, tag=f"kpT{ht}")
            nc.vector.tensor_copy(t, pk)
            kpT.append(t)

        qpTb = []
        for ht in range(HT):
            pj = pj_ps.tile([128, 128], F32, tag="pj")
            for it in range(IT):
                nc.tensor.matmul(
                    pj, lhsT=wqT[it][:, ht * 128:(ht + 1) * 128], rhs=queryT_sb[it],
                    start=(it == 0), stop=(it == IT - 1),
                )
            t = persist.tile([128, 128], F32, tag=f"qpTb{ht}")
            nc.vector.tensor_scalar_add(t, pj, bsum[ht])
            qpTb.append(t)

        # ---- main loop: tanh + v-reduction -----------------------------
        # scores are built transposed (PE output partition base must be
        # 32-aligned): scT_h[half][:, ks*64 + q%64] holds
        # scores[q, ks*128:(ks+1)*128].T for q in that 64-row half.
        # lhsT = tanh tile [h, k_sub] (stationary), rhs = v column
        # (moving), out = [k_sub, 1]. h-tile accumulation is contiguous per
        # column so the bank-wide has_written clearing of start=True never
        # hits an open group. Two half tiles live in different PSUM banks
        # so half-0 postprocessing overlaps the second half of the tanh
        # stream without PE-write/DVE-read bank collisions.
        QH = QROWS // 2
        scT_h = [sc_ps.tile([128, KT * QH], F32, name=f"scT{h}", tag=f"scT{h}")
                 for h in range(2)]

        state = {}

        def emit_half_post(half):
            """softmax + attn_w/attn_out for rows [half*QH, (half+1)*QH)."""
            value_sb = state["value"]
            r0 = half * QH
            # exp directly on the transposed PSUM scores with mask + the
            # static sum|v| bound folded into the per-partition bias:
            # eT[k, q] = exp(scT[k,q] + mcol[k] - M). eT is already in
            # matmul-lhsT layout, so attn_out and the row sums (ones
            # column) need no transposes at all.
            eT = []
            for ks in range(KT):
                d = persist.tile([128, QH], F32, tag=f"eT{half}_{ks}",
                                 name=f"eT{half}_{ks}")
                nc.scalar.activation(
                    d, scT_h[half][:, ks * QH:(ks + 1) * QH],
                    mybir.ActivationFunctionType.Exp,
                    bias=state["mbM"][:, ks:ks + 1],
                )
                eT.append(d)
            prs = tp_ps.tile([QH, 1], F32, tag="tp", name=f"prs{half}")
            for ks in range(KT):
                nc.tensor.matmul(prs, lhsT=eT[ks], rhs=state["one_col"],
                                 start=(ks == 0), stop=(ks == KT - 1))
            rinv = persist.tile([QH, 1], F32, tag=f"ri{half}", name=f"ri{half}")
            nc.vector.reciprocal(rinv, prs)

            # attn_out: h-halves so the first copy+DMA overlaps the second
            for hh in range(2):
                po = out_ps.tile([QH, H // 2], F32, tag="po", name=f"po{half}{hh}")
                for ks in range(KT):
                    nc.tensor.matmul(
                        po, lhsT=eT[ks],
                        rhs=value_sb[ks][:, hh * (H // 2):(hh + 1) * (H // 2)],
                        start=(ks == 0), stop=(ks == KT - 1),
                    )
                osb = persist.tile([QH, H // 2], F32, tag=f"osb{half}{hh}",
                                   name=f"osb{half}{hh}")
                nc.vector.tensor_scalar_mul(osb, po, rinv)
                nc.sync.dma_start(
                    out=attn_out[r0:r0 + QH, hh * (H // 2):(hh + 1) * (H // 2)],
                    in_=osb)

            # attn_w: transpose eT back to [q, k], normalize in the
            # PSUM eviction (off the attn_out critical path)
            paw = pj_ps.tile([QH, LK], F32, tag="pj", name=f"paw{half}")
            for ks in range(KT):
                nc.tensor.transpose(
                    paw[:, ks * 128:(ks + 1) * 128], eT[ks], ident,
                )
            aw = persist.tile([QH, LK], F32, tag=f"aw{half}", name=f"aw{half}")
            nc.vector.tensor_scalar_mul(aw, paw, rinv)
            # gpsimd queue: don't serialize behind the attn_out DMA on sync
            nc.gpsimd.dma_start(out=attn_w[r0:r0 + QH, :], in_=aw)

        # ramp-up block sizes: tiny first blocks so the tanh stream starts
        # as soon as kpT/qpTb land; steady-state blocks amortize overheads.
        BLOCKS = [1, 1, 2, 4, 8, 16, 32, 32, 16, 8, 8]
        assert sum(BLOCKS) == QROWS
        q0 = 0
        for blk, qb in enumerate(BLOCKS):
            if blk == 1:
                # emitted here so the scheduler runs these loads during the
                # main loop (off both the prologue and tail critical paths)
                vt_sb = persist.tile([128, 4 * H], F32)
                nc.sync.dma_start(out=vt_sb, in_=vt[:, :])
                state["value"] = [vt_sb[:, k * H:(k + 1) * H] for k in range(KT)]
                # mask as a single [1, LK] additive-bias row, folded into
                # the scores PSUM by rank-1 accumulate matmuls
                # mask as [128, KT] columns (k on partitions, one column
                # per k-subtile) -> additive bias in the scT layout
                mask_i = persist.tile([128, KT], I32)
                nc.sync.dma_start(
                    out=mask_i,
                    in_=bass.AP(tensor=mask, offset=0, ap=[[1, 128], [128, KT]]))
                mcol = persist.tile([128, KT], F32)
                # mask==1 -> 0.0 ; mask==0 -> NEG_BIG
                nc.vector.tensor_scalar(
                    out=mcol, in0=mask_i, scalar1=-NEG_BIG, scalar2=NEG_BIG,
                    op0=mybir.AluOpType.mult, op1=mybir.AluOpType.add,
                )
                state["mcol"] = mcol
                # softmax max-subtraction only needs an UPPER BOUND on the
                # scores; |scores| <= sum|v| since tanh is in (-1,1).
                # Compute once, broadcast to a per-partition column.
                one_col = persist.tile([128, 1], F32)
                nc.vector.memset(one_col, 1.0)
                vabs = persist.tile([128, 2], F32)
                for ht in range(HT):
                    nc.scalar.activation(
                        vabs[:, ht:ht + 1], cols_sb[:, 4 + ht:5 + ht],
                        mybir.ActivationFunctionType.Abs)
                pm = tp_ps.tile([1, 1], F32, tag="tp", name="pm")
                for ht in range(HT):
                    nc.tensor.matmul(pm, lhsT=vabs[:, ht:ht + 1], rhs=one_col,
                                     start=(ht == 0), stop=(ht == HT - 1))
                negm1 = persist.tile([1, 1], F32)
                nc.vector.tensor_scalar_mul(negm1, pm, -1.0)
                # broadcast the scalar to a per-partition column via a
                # K=1 matmul (engines cannot partition-broadcast SBUF)
                one_row = persist.tile([1, 128], F32)
                nc.vector.memset(one_row, 1.0)
                pb = tp_ps.tile([128, 1], F32, tag="tp", name="pb")
                nc.tensor.matmul(pb, lhsT=one_row, rhs=negm1,
                                 start=True, stop=True)
                negm_col = persist.tile([128, 1], F32)
                nc.vector.tensor_copy(negm_col, pb)
                # mbM[k, ks] = mask_bias[k, ks] - M : the complete exp bias
                mbM = persist.tile([128, KT], F32)
                nc.vector.tensor_scalar_add(mbM, mcol, negm_col)
                state["mbM"] = mbM
                state["one_col"] = one_col
            tbs = []
            for ht in range(HT):
                sin = sin_pool.tile([128, qb * LK], TDT, tag=f"sin{ht}",
                                    name=f"sin{ht}")
                for j in range(qb):
                    q = q0 + j
                    nc.vector.tensor_scalar_add(
                        sin[:, j * LK:(j + 1) * LK], kpT[ht], qpTb[ht][:, q:q + 1],
                    )
                # in-place tanh: ACTIVATE reads each element before writing
                # it, so out==in is safe and halves the big-tile SBUF
                nc.scalar.activation(sin, sin, mybir.ActivationFunctionType.Tanh)
                tbs.append(sin)
            for j in range(qb):
                q = q0 + j
                half, ql = q // QH, q % QH
                for ks in range(KT):
                    col = ks * QH + ql
                    for ht in range(HT):
                        nc.tensor.matmul(
                            scT_h[half][:, col:col + 1],
                            lhsT=tbs[ht][:, j * LK + ks * 128:j * LK + (ks + 1) * 128],
                            rhs=v_col[ht],
                            start=(ht == 0), stop=(ht == HT - 1),
                        )
            q0 += qb
            if q0 == QH:
                emit_half_post(0)
        emit_half_post(1)

    nc.compile()
    return nc


def get_nc():
    if "nc" not in _CACHE:
        _CACHE["nc"] = _build_nc()
    return _CACHE["nc"]


def make_in_maps(query, key, value, mask, Wq, bq, Wk, bk, v, bv=None):
    query = np.ascontiguousarray(np.asarray(query, dtype=np.float32))
    key = np.ascontiguousarray(np.asarray(key, dtype=np.float32))
    value = np.ascontiguousarray(np.asarray(value, dtype=np.float32))
    mask = np.ascontiguousarray(np.asarray(mask, dtype=np.int32))
    Wq = np.ascontiguousarray(np.asarray(Wq, dtype=np.float32))
    bq = np.ascontiguousarray(np.asarray(bq, dtype=np.float32))
    Wk = np.ascontiguousarray(np.asarray(Wk, dtype=np.float32))
    bk = np.ascontiguousarray(np.asarray(bk, dtype=np.float32))
    v = np.ascontiguousarray(np.asarray(v, dtype=np.float32))

    from concourse import mybir as _mybir
    bf16 = _mybir.dt.np(TDT)

    WqT = Wq.T
    WkT = Wk.T
    # wt = (WqT0 | WqT1 | WkT0 | WkT1), each [128, H]
    wt = np.ascontiguousarray(np.concatenate(
        [WqT[:128], WqT[128:], WkT[:128], WkT[128:]], axis=1)).astype(bf16)
    # cols = (bq0|bq1|bk0|bk1|v0|v1)
    cols = np.ascontiguousarray(np.stack(
        [bq[:128], bq[128:], bk[:128], bk[128:], v[:128], v[128:]], axis=1))

    kt_b = {}
    vt_b = {}
    for b in range(B):
        keyT = key[b].T  # [H, LK]
        kt_b[b] = np.ascontiguousarray(np.concatenate(
            [keyT[:128], keyT[128:]], axis=1)).astype(bf16)
        vt_b[b] = np.ascontiguousarray(np.concatenate(
            [value[b, k * 128:(k + 1) * 128, :] for k in range(4)], axis=1))

    in_maps = []
    for c in range(NCORES):
        b = c // 2
        r0 = (c % 2) * QROWS
        qT = query[b, r0:r0 + QROWS, :].T  # [H, QROWS]
        qt = np.ascontiguousarray(
            np.concatenate([qT[:128], qT[128:]], axis=1)).astype(bf16)
        in_maps.append({
            "qt": qt,
            "kt": kt_b[b],
            "vt": vt_b[b],
            "mask": mask[b],
            "wt": wt,
            "cols": cols,
        })
    return in_maps


def assemble(results):
    attn_out = np.empty((B, LQ, H), dtype=np.float32)
    attn_w = np.empty((B, LQ, LK), dtype=np.float32)
    for c in range(NCORES):
        b = c // 2
        r0 = (c % 2) * QROWS
        attn_out[b, r0:r0 + QROWS, :] = results[c]["attn_out"]
        attn_w[b, r0:r0 + QROWS, :] = results[c]["attn_w"]
    return attn_out, attn_w


def kernel(query, key, value, mask, Wq, bq, Wk, bk, v, bv=None):
    from concourse.bass_utils import run_bass_kernel_spmd

    nc = get_nc()
    in_maps = make_in_maps(query, key, value, mask, Wq, bq, Wk, bk, v, bv)
    res = run_bass_kernel_spmd(nc, in_maps, core_ids=list(range(NCORES)))
    return assemble(res.results)


# revision 78
# speedup vs baseline: 1.0056x; 1.0056x over previous
"""Additive (Bahdanau) attention on 8 TRN2 NeuronCores.

reference:
    q = query @ Wq.T + bq                  [B, Lq, H]
    k = key @ Wk.T + bk                    [B, Lk, H]
    scores[b,q,k] = sum_h v[h] * tanh(qp[b,q,h] + kp[b,k,h]) (+ bv)
    scores = where(mask==0, -inf, scores)
    attn_w = softmax(scores, axis=-1)      [B, Lq, Lk]
    attn_out = attn_w @ value              [B, Lq, H]
    returns (attn_out, attn_w)

Sharding: B*Lq = 1024 query rows split 8 ways -> 128 rows/core, each core
gets its batch's key/value/mask. Zero cross-core communication.

The workload is bound by ScalarE (ACT), the only transcendental engine:
Lq*Lk*H/8 = 16.8M tanh evaluations per core at 1 elem/lane/cycle @ 1.2
GHz = ~109 us. Everything else is pipelined under that stream.

Per-core dataflow (h on partitions for the tanh pipeline):
  - host packs transposed bf16 layouts (queryT/keyT/WqT/WkT concatenated
    into one array per DMA) — pure layout/rounding prep, all math stays
    on device; PE computes projections qpT[h,q], kpT[h,k] (f32 PSUM).
  - main loop over ramped q-blocks x h-tiles: DVE tensor_scalar_add
    broadcasts qpT[:,q] over kpT (bf16, 4x mode) -> big S_in tile;
    ScalarE tanh in [128, qb*512] instructions; PE reduces over h with
    the v column as moving operand, tanh tile as stationary (FWL bf16),
    giving transposed scores scT[k, q] in PSUM (PE output partition base
    must be 32-aligned, which forbids per-q score rows directly).
  - scores are built in two 64-row half tiles in different PSUM banks;
    each half's postprocessing (PSUM eviction fused with the mask bias
    via tensor_scalar — the mask is per-partition in the transposed
    layout — PE transposes to [q, k], masked softmax with exp accum_out
    row sums, attn_w DMA, PE re-transpose, attn_out matmul split in two
    h-halves) overlaps the other half's tanh stream. The softmax
    max-subtraction uses the static bound sum|v| >= |scores| (tanh < 1),
    so exp needs no row-max reduction; attn_out is computed from the
    unnormalized exp with 1/rowsum folded into the PSUM eviction.

Numerics: the tanh/score pipeline runs in bf16 (inputs rounded on host,
tanh output bf16, f32 PSUM accumulation); softmax and attn_out are f32.
End-to-end rel err ~5e-3. bv is mathematically irrelevant: softmax is
shift invariant and raw scores are not returned.
"""

from contextlib import ExitStack

import numpy as np

import concourse.bass as bass
import concourse.tile as tile
from concourse import bacc, mybir
from concourse.masks import make_identity

B, LQ, LK, H = 4, 256, 512, 256
NCORES = 8
QROWS = B * LQ // NCORES  # 128 query rows per core
NEG_BIG = -1e30

F32 = mybir.dt.float32
BF16 = mybir.dt.bfloat16
I32 = mybir.dt.int32
# dtype of the tanh pipeline (kpT/qpTb/sin/tb/v): bf16 gives DVE 4x mode
# on the broadcast adds and halves SBUF; scores accumulate in f32 PSUM.
TDT = BF16

_CACHE: dict = {}


def _build_nc():
    nc = bacc.Bacc()

    # host-side layout prep (make_in_maps): transposed + concatenated into
    # one wide array per DMA so the prologue is a handful of transfers.
    #   wt:   (WqT0 | WqT1 | WkT0 | WkT1)          [128, 4*H]
    #   qt:   (queryT0 | queryT1)                  [128, 2*QROWS]
    #   kt:   (keyT0 | keyT1)                      [128, 2*LK]
    #   vt:   (value[0:128] | ... | value[384:512])[128, 4*H]
    #   cols: (bq0|bq1|bk0|bk1|v0|v1)              [128, 6]
    wt = nc.declare_dram_parameter("wt", [128, 4 * H], TDT, isOutput=False)
    qt = nc.declare_dram_parameter("qt", [128, 2 * QROWS], TDT, isOutput=False)
    kt = nc.declare_dram_parameter("kt", [128, 2 * LK], TDT, isOutput=False)
    vt = nc.declare_dram_parameter("vt", [128, 4 * H], F32, isOutput=False)
    cols = nc.declare_dram_parameter("cols", [128, 6], F32, isOutput=False)
    mask = nc.declare_dram_parameter("mask", [LK], I32, isOutput=False)

    attn_out = nc.declare_dram_parameter("attn_out", [QROWS, H], F32, isOutput=True)
    attn_w = nc.declare_dram_parameter("attn_w", [QROWS, LK], F32, isOutput=True)

    HT = H // 128  # h tiles (2)
    IT = H // 128  # hin tiles (2)
    KT = LK // 128  # key row tiles (4)


    with tile.TileContext(nc) as tc, ExitStack() as ctx:
        persist = ctx.enter_context(tc.tile_pool(name="persist", bufs=1))
        sin_pool = ctx.enter_context(tc.tile_pool(name="sin", bufs=2))

        tp_ps = ctx.enter_context(tc.tile_pool(name="tp_ps", bufs=2, space="PSUM"))
        pj_ps = ctx.enter_context(tc.tile_pool(name="pj_ps", bufs=2, space="PSUM"))
        sc_ps = ctx.enter_context(tc.tile_pool(name="sc_ps", bufs=1, space="PSUM"))
        out_ps = ctx.enter_context(tc.tile_pool(name="out_ps", bufs=2, space="PSUM"))

        # ---- packed loads (one DMA each; critical path first) ----------
        # key halves + weights on three queues so all prologue-critical
        # bytes land in parallel; the ACT-queued half is issued before the
        # warm tanh so it isn't delayed by the activation-table load
        kt_sb = persist.tile([128, 2 * LK], TDT)
        nc.sync.dma_start(out=kt_sb[:, :LK], in_=kt[:, :LK])
        nc.scalar.dma_start(out=kt_sb[:, LK:], in_=kt[:, LK:])
        wt_sb = persist.tile([128, 4 * H], TDT)
        nc.gpsimd.dma_start(out=wt_sb, in_=wt[:, :])

        # ---- constants / identity / ACT table warmup -------------------
        ident = persist.tile([128, 128], F32)
        make_identity(nc, ident)
        warm = persist.tile([128, 1], F32)
        nc.vector.memset(warm, 0.0)
        # touch the exp_and_others table set early (covers tanh + exp)
        nc.scalar.activation(warm, warm, mybir.ActivationFunctionType.Tanh)
        cols_sb = persist.tile([128, 6], F32)
        nc.sync.dma_start(out=cols_sb, in_=cols[:, :])
        qt_sb = persist.tile([128, 2 * QROWS], TDT)
        nc.sync.dma_start(out=qt_sb, in_=qt[:, :])

        wqT = [wt_sb[:, it * H:(it + 1) * H] for it in range(IT)]
        wkT = [wt_sb[:, (2 + it) * H:(3 + it) * H] for it in range(IT)]
        queryT_sb = [qt_sb[:, it * QROWS:(it + 1) * QROWS] for it in range(IT)]
        keyT_sb = [kt_sb[:, it * LK:(it + 1) * LK] for it in range(IT)]
        bq_col = [cols_sb[:, ht:ht + 1] for ht in range(HT)]
        bk_col = [cols_sb[:, 2 + ht:3 + ht] for ht in range(HT)]

        # v columns: bf16 to match the tanh-tile lhsT dtype
        v_col = []
        for ht in range(HT):
            t = persist.tile([128, 1], TDT, tag=f"v{ht}")
            nc.vector.tensor_copy(t, cols_sb[:, 4 + ht:5 + ht])
            v_col.append(t)
        bsum = []
        for ht in range(HT):
            t = persist.tile([128, 1], F32, tag=f"bsum{ht}")
            nc.vector.tensor_add(t, bq_col[ht], bk_col[ht])
            bsum.append(t)

        # ---- projections ----------------------------------------------
        # qpT[ht][h', qrow] = sum_hin Wq[h, hin] * queryT[hin, qrow]  (+bq+bk)
        # kpT first: it is the long pole into the first tanh block.
        # ScalarE does the PSUM->SBUF copies (ACT is idle in the prologue,
        # DVE is busy with the first adds).
        kpT = []
        for ht in range(HT):
            pk = pj_ps.tile([128, LK], F32, tag="pj")
            for it in range(IT):
                nc.tensor.matmul(
                    pk, lhsT=wkT[it][:, ht * 128:(ht + 1) * 128], rhs=keyT_sb[it],
                    start=(it == 0), stop=(it == IT - 1),
                )
            t = persist.tile([128, LK], TDT, tag=f"kpT{ht}")
            nc.scalar.copy(t, pk)
            kpT.append(t)

        qpTb = []
        for ht in range(HT):
            pj = pj_ps.tile([128, 128], F32, tag="pj")
            for it in range(IT):
                nc.tensor.matmul(
                    pj, lhsT=wqT[it][:, ht * 128:(ht + 1) * 128], rhs=queryT_sb[it],
                    start=(it == 0), stop=(it == IT - 1),
                )
            t = persist.tile([128, 128], F32, tag=f"qpTb{ht}")
            nc.vector.tensor_scalar_add(t, pj, bsum[ht])
            qpTb.append(t)

        # ---- main loop: tanh + v-reduction -----------------------------
        # scores are built transposed (PE output partition base must be
        # 32-aligned): scT_h[half][:, ks*64 + q%64] holds
        # scores[q, ks*128:(ks+1)*128].T for q in that 64-row half.
        # lhsT = tanh tile [h, k_sub] (stationary), rhs = v column
        # (moving), out = [k_sub, 1]. h-tile accumulation is contiguous per
        # column so the bank-wide has_written clearing of start=True never
        # hits an open group. Two half tiles live in different PSUM banks
        # so half-0 postprocessing overlaps the second half of the tanh
        # stream without PE-write/DVE-read bank collisions.
        QH = QROWS // 2
        scT_h = [sc_ps.tile([128, KT * QH], F32, name=f"scT{h}", tag=f"scT{h}")
                 for h in range(2)]

        state = {}

        def emit_half_post(half):
            """softmax + attn_w/attn_out for rows [half*QH, (half+1)*QH)."""
            value_sb = state["value"]
            r0 = half * QH
            # exp directly on the transposed PSUM scores with mask + the
            # static sum|v| bound folded into the per-partition bias:
            # eT[k, q] = exp(scT[k,q] + mcol[k] - M). eT is already in
            # matmul-lhsT layout, so attn_out and the row sums (ones
            # column) need no transposes at all.
            eT = []
            for ks in range(KT):
                d = persist.tile([128, QH], F32, tag=f"eT{half}_{ks}",
                                 name=f"eT{half}_{ks}")
                nc.scalar.activation(
                    d, scT_h[half][:, ks * QH:(ks + 1) * QH],
                    mybir.ActivationFunctionType.Exp,
                    bias=state["mbM"][:, ks:ks + 1],
                )
                eT.append(d)
            prs = tp_ps.tile([QH, 1], F32, tag="tp", name=f"prs{half}")
            for ks in range(KT):
                nc.tensor.matmul(prs, lhsT=eT[ks], rhs=state["one_col"],
                                 start=(ks == 0), stop=(ks == KT - 1))
            rinv = persist.tile([QH, 1], F32, tag=f"ri{half}", name=f"ri{half}")
            nc.vector.reciprocal(rinv, prs)

            # attn_out: h-halves so the first copy+DMA overlaps the second
            for hh in range(2):
                po = out_ps.tile([QH, H // 2], F32, tag="po", name=f"po{half}{hh}")
                for ks in range(KT):
                    nc.tensor.matmul(
                        po, lhsT=eT[ks],
                        rhs=value_sb[ks][:, hh * (H // 2):(hh + 1) * (H // 2)],
                        start=(ks == 0), stop=(ks == KT - 1),
                    )
                osb = persist.tile([QH, H // 2], F32, tag=f"osb{half}{hh}",
                                   name=f"osb{half}{hh}")
                nc.vector.tensor_scalar_mul(osb, po, rinv)
                nc.sync.dma_start(
                    out=attn_out[r0:r0 + QH, hh * (H // 2):(hh + 1) * (H // 2)],
                    in_=osb)

            # attn_w: transpose eT back to [q, k], normalize in the
            # PSUM eviction (off the attn_out critical path)
            paw = pj_ps.tile([QH, LK], F32, tag="pj", name=f"paw{half}")
            for ks in range(KT):
                nc.tensor.transpose(
                    paw[:, ks * 128:(ks + 1) * 128], eT[ks], ident,
                )
            aw = persist.tile([QH, LK], F32, tag=f"aw{half}", name=f"aw{half}")
            nc.vector.tensor_scalar_mul(aw, paw, rinv)
            # gpsimd queue: don't serialize behind the attn_out DMA on sync
            nc.gpsimd.dma_start(out=attn_w[r0:r0 + QH, :], in_=aw)

        # ramp-up block sizes: tiny first blocks so the tanh stream starts
        # as soon as kpT/qpTb land; steady-state blocks amortize overheads.
        BLOCKS = [1, 1, 2, 4, 8, 16, 32, 32, 16, 8, 8]
        assert sum(BLOCKS) == QROWS
        q0 = 0
        for blk, qb in enumerate(BLOCKS):
            if blk == 1:
                # emitted here so the scheduler runs these loads during the
                # main loop (off both the prologue and tail critical paths)
                vt_sb = persist.tile([128, 4 * H], F32)
                nc.sync.dma_start(out=vt_sb, in_=vt[:, :])
                state["value"] = [vt_sb[:, k * H:(k + 1) * H] for k in range(KT)]
                # mask as a single [1, LK] additive-bias row, folded into
                # the scores PSUM by rank-1 accumulate matmuls
                # mask as [128, KT] columns (k on partitions, one column
                # per k-subtile) -> additive bias in the scT layout
                mask_i = persist.tile([128, KT], I32)
                nc.sync.dma_start(
                    out=mask_i,
                    in_=bass.AP(tensor=mask, offset=0, ap=[[1, 128], [128, KT]]))
                mcol = persist.tile([128, KT], F32)
                # mask==1 -> 0.0 ; mask==0 -> NEG_BIG
                nc.vector.tensor_scalar(
                    out=mcol, in0=mask_i, scalar1=-NEG_BIG, scalar2=NEG_BIG,
                    op0=mybir.AluOpType.mult, op1=mybir.AluOpType.add,
                )
                state["mcol"] = mcol
                # softmax max-subtraction only needs an UPPER BOUND on the
                # scores; |scores| <= sum|v| since tanh is in (-1,1).
                # Compute once, broadcast to a per-partition column.
                one_col = persist.tile([128, 1], F32)
                nc.vector.memset(one_col, 1.0)
                vabs = persist.tile([128, 2], F32)
                for ht in range(HT):
                    nc.scalar.activation(
                        vabs[:, ht:ht + 1], cols_sb[:, 4 + ht:5 + ht],
                        mybir.ActivationFunctionType.Abs)
                pm = tp_ps.tile([1, 1], F32, tag="tp", name="pm")
                for ht in range(HT):
                    nc.tensor.matmul(pm, lhsT=vabs[:, ht:ht + 1], rhs=one_col,
                                     start=(ht == 0), stop=(ht == HT - 1))
                negm1 = persist.tile([1, 1], F32)
                nc.vector.tensor_scalar_mul(negm1, pm, -1.0)
                # broadcast the scalar to a per-partition column via a
                # K=1 matmul (engines cannot partition-broadcast SBUF)
                one_row = persist.tile([1, 128], F32)
                nc.vector.memset(one_row, 1.0)
                pb = tp_ps.tile([128, 1], F32, tag="tp", name="pb")
                nc.tensor.matmul(pb, lhsT=one_row, rhs=negm1,
                                 start=True, stop=True)
                negm_col = persist.tile([128, 1], F32)
                nc.vector.tensor_copy(negm_col, pb)
                # mbM[k, ks] = mask_bias[k, ks] - M : the complete exp bias
                mbM = persist.tile([128, KT], F32)
                nc.vector.tensor_scalar_add(mbM, mcol, negm_col)
                state["mbM"] = mbM
                state["one_col"] = one_col
            tbs = []
            for ht in range(HT):
                sin = sin_pool.tile([128, qb * LK], TDT, tag=f"sin{ht}",
                                    name=f"sin{ht}")
                for j in range(qb):
                    q = q0 + j
                    nc.vector.tensor_scalar_add(
                        sin[:, j * LK:(j + 1) * LK], kpT[ht], qpTb[ht][:, q:q + 1],
                    )
                # in-place tanh: ACTIVATE reads each element before writing
                # it, so out==in is safe and halves the big-tile SBUF
                nc.scalar.activation(sin, sin, mybir.ActivationFunctionType.Tanh)
                tbs.append(sin)
            for j in range(qb):
                q = q0 + j
                half, ql = q // QH, q % QH
                for ks in range(KT):
                    col = ks * QH + ql
                    for ht in range(HT):
                        nc.tensor.matmul(
                            scT_h[half][:, col:col + 1],
                            lhsT=tbs[ht][:, j * LK + ks * 128:j * LK + (ks + 1) * 128],
                            rhs=v_col[ht],
                            start=(ht == 0), stop=(ht == HT - 1),
                        )
            q0 += qb
            if q0 == QH:
                emit_half_post(0)
        emit_half_post(1)

    nc.compile()
    return nc


def get_nc():
    if "nc" not in _CACHE:
        _CACHE["nc"] = _build_nc()
    return _CACHE["nc"]


def make_in_maps(query, key, value, mask, Wq, bq, Wk, bk, v, bv=None):
    query = np.ascontiguousarray(np.asarray(query, dtype=np.float32))
    key = np.ascontiguousarray(np.asarray(key, dtype=np.float32))
    value = np.ascontiguousarray(np.asarray(value, dtype=np.float32))
    mask = np.ascontiguousarray(np.asarray(mask, dtype=np.int32))
    Wq = np.ascontiguousarray(np.asarray(Wq, dtype=np.float32))
    bq = np.ascontiguousarray(np.asarray(bq, dtype=np.float32))
    Wk = np.ascontiguousarray(np.asarray(Wk, dtype=np.float32))
    bk = np.ascontiguousarray(np.asarray(bk, dtype=np.float32))
    v = np.ascontiguousarray(np.asarray(v, dtype=np.float32))

    from concourse import mybir as _mybir
    bf16 = _mybir.dt.np(TDT)

    WqT = Wq.T
    WkT = Wk.T
    # wt = (WqT0 | WqT1 | WkT0 | WkT1), each [128, H]
    wt = np.ascontiguousarray(np.concatenate(
        [WqT[:128], WqT[128:], WkT[:128], WkT[128:]], axis=1)).astype(bf16)
    # cols = (bq0|bq1|bk0|bk1|v0|v1)
    cols = np.ascontiguousarray(np.stack(
        [bq[:128], bq[128:], bk[:128], bk[128:], v[:128], v[128:]], axis=1))

    kt_b = {}
    vt_b = {}
    for b in range(B):
        keyT = key[b].T  # [H, LK]
        kt_b[b] = np.ascontiguousarray(np.concatenate(
            [keyT[:128], keyT[128:]], axis=1)).astype(bf16)
        vt_b[b] = np.ascontiguousarray(np.concatenate(
            [value[b, k * 128:(k + 1) * 128, :] for k in range(4)], axis=1))

    in_maps = []
    for c in range(NCORES):
        b = c // 2
        r0 = (c % 2) * QROWS
        qT = query[b, r0:r0 + QROWS, :].T  # [H, QROWS]
        qt = np.ascontiguousarray(
            np.concatenate([qT[:128], qT[128:]], axis=1)).astype(bf16)
        in_maps.append({
            "qt": qt,
            "kt": kt_b[b],
            "vt": vt_b[b],
            "mask": mask[b],
            "wt": wt,
            "cols": cols,
        })
    return in_maps


def assemble(results):
    attn_out = np.empty((B, LQ, H), dtype=np.float32)
    attn_w = np.empty((B, LQ, LK), dtype=np.float32)
    for c in range(NCORES):
        b = c // 2
        r0 = (c % 2) * QROWS
        attn_out[b, r0:r0 + QROWS, :] = results[c]["attn_out"]
        attn_w[b, r0:r0 + QROWS, :] = results[c]["attn_w"]
    return attn_out, attn_w


def kernel(query, key, value, mask, Wq, bq, Wk, bk, v, bv=None):
    from concourse.bass_utils import run_bass_kernel_spmd

    nc = get_nc()
    in_maps = make_in_maps(query, key, value, mask, Wq, bq, Wk, bk, v, bv)
    res = run_bass_kernel_spmd(nc, in_maps, core_ids=list(range(NCORES)))
    return assemble(res.results)
